# revision 1
# baseline (speedup 1.0000x reference)
"""T5-style encoder self-attention (dense_transformer) on 8 Trainium2 NeuronCores.

Problem (full shapes): hidden [2,2048,2048], Wq/Wk/Wv/Wo [2048,2048],
rel_emb [32,32] (bidirectional T5 relative-position bias), mask [2,1,1,2048].

Sharding: data-parallel over batch (2) x tensor-parallel over heads (4 groups
of 8 heads) = 8 cores, Megatron-style. Each core computes a partial output
[2048,2048] for its batch (its 8 heads through its Wo row-slice); the host
sums 4 partials per batch.

Per-core kernel design (bf16 operands, fp32 PSUM accumulation):
  - Both relative-position bias diagonal tables are HOST-computed (they are
    data-dependent only through rel_emb, a [32,32] input): brel = 8*bias
    (log domain, for additive injection) and erel = exp(bias) (for
    multiplicative application), each [8 heads, 4096 diagonals] bf16.
  - Phase B: single pass over x^T computes pair-0 Q^T/K^T and V for ALL
    heads (6 matmuls per x^T tile, PE-bound).  Q^T is stored with s
    REVERSED so the bias becomes a positive-shear Toeplitz.
  - Phase C attention, per (head-pair, q-chunk), k-tile loop pipelined one
    iteration ahead:
      * near-diagonal k-tiles (|k-q| < ~91 somewhere in tile): the bias tile
        is INJECTED into PSUM via an identity matmul (start=True), then the
        two packed QK matmuls accumulate on top; ACT computes
        exp(s/8 + mask + bias) in one shot - no DVE multiply.
      * far k-tiles: bias is exactly constant (bucket 15/31 saturates), but
        rather than bake runtime constants we keep the baseline DVE multiply
        with the erel shear tile (DVE is otherwise idle).
      * next-pair Q/K projection matmuls are interleaved PER k-tile so they
        fill the PE's ACT-wait bubbles (the in-order PE queue can only run
        work that is already emitted ahead of the blocked PV matmul).
  - V augmentation: per pair, even head block = [v(0:64) | ones(64)] (M=65,
    denominator lands on PSUM partition 64), odd head block = 128 wide with
    ones at col 32 and v at cols 64:128 (denominator on partition 32, ctx on
    partitions 64:128), keeping every normalize op partition-aligned.
  - Normalize is DEFERRED and PE-free: cx evacuates to SBUF at qc end
    (freeing its PSUM slot), then one qc later a DVE+DMA-only chain runs:
    pack denominator rows to a base-0 tile (custom DVE ops require base
    partition 0), reciprocal_approx_fast, bounce the two reciprocal rows
    through DRAM, stride-0 DMAs broadcast them across partitions, and fused
    DVE tensor_tensors do normalize + un-reverse + bf16 writeback.
  - The next qc's first score-group is pre-emitted in the current qc's tail
    (exactly one PSUM slot is free there) so ACT never idles at boundaries.
  - DMA queue plan (1KB-packet rate ~85 GB/s/queue is the constraint):
    weights host-shuffled to partition-contiguous layout and chunk-loaded on
    the gpsimd queue; x^T tiles split across sync+gpsimd queues with
    per-queue tile pools (cross-queue shared pools head-of-line block);
    u tables quarter-loaded per qc.
  - Phase D output projection: descending s-tiles (low tiles depend on the
    last deferred normalize), m looped inside nd so consecutive matmuls hit
    different PSUM banks; evacuation alternates ACT/DVE.
"""

import math
import sys

for _p in ("/opt/trn_rl_repo",):
    if _p not in sys.path:
        sys.path.insert(0, _p)

import numpy as np

import concourse.bass as bass
import concourse.mybir as mybir
import concourse.tile as tile
from concourse import bacc
from concourse.bass_utils import run_bass_kernel_spmd

DT = mybir.dt
AF = mybir.ActivationFunctionType
OP = mybir.AluOpType

# ---- problem constants (hardcoded per contract) ----
B, S, D = 2, 2048, 2048
N_HEADS, D_KV = 32, 64
NUM_BUCKETS, MAX_DISTANCE = 32, 128
NCORES = 8
HL = 8            # heads per core
P = 128
SC = 512          # free-dim chunk
NKT = S // P      # 16 k-tiles
NQC = S // SC     # 4 q-chunks
NDT = D // P      # 16 D-tiles
NMT = (HL * D_KV) // P   # 4 hd m-tiles per core
NPAIR = HL // 2   # 4 head pairs per core
W_U = 2944        # far-region exp-table shear width (diagonals 512..3456)
U_OFF = 512       # first diagonal index covered by the u shear tiles
W_NEAR = 1152     # near-window raw-table shear tile width
NDIAG = 4096
VW = 193          # vaug per-(kt,pair) width: even block 65 + odd block 128

# near-tile bookkeeping: tile (kt, qc) is "far" iff |k-q| >= 91 everywhere
def _is_near(kt, qc):
    dmin = 128 * kt - 512 * qc - 511
    dmax = 128 * kt + 127 - 512 * qc
    return not (dmin >= 91 or dmax <= -91)

NEAR = {(kt, qc): _is_near(kt, qc) for kt in range(NKT) for qc in range(NQC)}
# raw-table window base per qc (clamped so the shear read stays in bounds)
B0 = [max(0, 1024 * qc - 128) for qc in range(NQC)]
WQC = [min(W_NEAR, NDIAG - 127 - b0) for b0 in B0]


def _rel_bucket_host(d):
    """Exact numpy replica of reference._relative_position_bucket."""
    num_buckets = NUM_BUCKETS // 2          # 16
    max_exact = num_buckets // 2            # 8
    rel = np.asarray(d, dtype=np.int64)
    buckets = (rel > 0).astype(np.int32) * num_buckets
    arel = np.abs(rel)
    is_small = arel < max_exact
    rp_safe = np.maximum(arel, 1).astype(np.float32)
    log_ratio = np.log(rp_safe / np.float32(max_exact)).astype(np.float32)
    scale = np.float32(math.log(MAX_DISTANCE / max_exact))
    rp_large = max_exact + (log_ratio / scale * np.float32(num_buckets - max_exact)).astype(np.int32)
    rp_large = np.minimum(rp_large, num_buckets - 1)
    buckets = buckets + np.where(is_small, arel.astype(np.int32), rp_large)
    return buckets.astype(np.int32)


def _bias_tables(rel_emb_slice):
    """rel_emb_slice: [NUM_BUCKETS, HL] fp32 -> (brel, erel) [HL, NDIAG].
    brel[h, i] = 8 * bias(d = i - 2047); erel[h, i] = exp(bias)."""
    import ml_dtypes
    i = np.arange(NDIAG - 1)
    b = _rel_bucket_host(i - (S - 1))                  # [4095]
    vals = rel_emb_slice[b, :]                         # [4095, HL] fp32
    brel = np.zeros((HL, NDIAG), dtype=np.float32)
    erel = np.zeros((HL, NDIAG), dtype=np.float32)
    brel[:, : NDIAG - 1] = 8.0 * vals.T
    erel[:, : NDIAG - 1] = np.exp(vals.T)
    return (brel.astype(ml_dtypes.bfloat16), erel.astype(ml_dtypes.bfloat16))


def _build():
    nc = bacc.Bacc(None, name="attn_tp")

    xt = nc.declare_dram_parameter("xt", [D, S], DT.bfloat16, isOutput=False)
    # weights arrive HOST-SHUFFLED to [p][kt][h] so per-partition lines are
    # contiguous multi-KB runs (DMA packet rate is the limiter at 1KB lines)
    wq = nc.declare_dram_parameter("wq", [P, NDT * HL * D_KV], DT.bfloat16, isOutput=False)
    wk = nc.declare_dram_parameter("wk", [P, NDT * HL * D_KV], DT.bfloat16, isOutput=False)
    wv = nc.declare_dram_parameter("wv", [P, NDT * HL * D_KV], DT.bfloat16, isOutput=False)
    wo = nc.declare_dram_parameter("wo", [P, NMT * D], DT.bfloat16, isOutput=False)
    mask = nc.declare_dram_parameter("mask", [S], DT.float32, isOutput=False)
    brel = nc.declare_dram_parameter("brel", [HL, NDIAG], DT.bfloat16, isOutput=False)
    erel = nc.declare_dram_parameter("erel", [HL, NDIAG], DT.bfloat16, isOutput=False)
    ident = nc.declare_dram_parameter("ident", [P, P], DT.bfloat16, isOutput=False)
    out = nc.declare_dram_parameter("out", [S, D], DT.float32, isOutput=True)

    with tile.TileContext(nc) as tc:
        with (
            tc.tile_pool(name="res", bufs=1) as res,          # persistent tensors
            tc.tile_pool(name="xtp", bufs=4) as xtp,          # x^T tiles (sync q)
            tc.tile_pool(name="xtpc", bufs=4) as xtpc,        # x^T tiles (gpsimd q)
            tc.tile_pool(name="upool", bufs=3) as upool,      # exp-bias shear tiles
            tc.tile_pool(name="urawp", bufs=2) as urawp,      # raw-bias near windows
            tc.tile_pool(name="pexp", bufs=4) as pexpp,       # probs tiles
            tc.tile_pool(name="stage", bufs=2) as stage,      # normalize staging
            tc.tile_pool(name="outp", bufs=3) as outp,        # out staging
            tc.tile_pool(name="psum", bufs=4, space="PSUM") as psum,  # [128,1024] slots
            tc.tile_pool(name="dram", bufs=2, space="DRAM") as dramp,
        ):
            # ---------- constants ----------
            mask_sb = res.tile([P, NKT], DT.float32, tag="mask")
            nc.sync.dma_start(mask_sb[:], mask.ap().rearrange("(kt p) -> p kt", p=P))

            id_sb = res.tile([P, P], DT.bfloat16, tag="ident")
            nc.sync.dma_start(id_sb[:], ident[:])

            # weights (resident, bf16).  wq/wk/wv stream in per-kd chunks on
            # side DMA queues so the xt stream is not blocked at startup; wo
            # loads once on the (idle) scalar queue.
            wq_sb = res.tile([P, NDT, HL * D_KV], DT.bfloat16, tag="wq")
            wk_sb = res.tile([P, NDT, HL * D_KV], DT.bfloat16, tag="wk")
            wv_sb = res.tile([P, NDT, HL * D_KV], DT.bfloat16, tag="wv")
            wo_sb = res.tile([P, NMT, D], DT.bfloat16, tag="wo")

            # persistent activations
            qt_sb = res.tile([P, NMT, S], DT.bfloat16, tag="qt")   # q REVERSED
            kt_sb = res.tile([P, NMT, S], DT.bfloat16, tag="kt")
            vaug = res.tile([P, NKT, NPAIR, VW], DT.bfloat16, tag="vaug")
            ctxt = res.tile([P, NMT, S], DT.bfloat16, tag="ctxt")
            # only the two ones-columns are ever read outside the V blocks
            # (psum rows other than the denominator rows are never consumed)
            nc.vector.memset(vaug[:, :, :, 64:65], 1.0)
            nc.vector.memset(vaug[:, :, :, 97:98], 1.0)

            # ACT exp table warm-up (hide the ~2.7us table load under phase B)
            warm = res.tile([1, 2], DT.float32, tag="warm")
            nc.scalar.activation(out=warm[0:1, 0:1], in_=mask_sb[0:1, 0:1], func=AF.Exp)

            def rev_ap(base, jg0):
                """reversed-q view: base is a [rows, S] AP slice of a res
                tensor; returns [rows, SC] AP walking q backwards so writing
                reversed data lands in natural order."""
                return bass.AP(
                    tensor=base.tensor,
                    offset=base.offset + (S - 1 - jg0),
                    ap=[list(base.ap[0]), [-1, SC]],
                )

            UQ = W_U // 4
            def load_u(pr, quarter=None, u=None):
                """full exp-table shear tile [P, 2, W_U] for pair pr.
                quarter=None loads everything; otherwise loads one quarter of
                each head's span into the passed tile (spreads the 2 MB burst
                across the previous pair's four q-chunks)."""
                if u is None:
                    u = upool.tile([P, 2, W_U], DT.bfloat16, tag="u",
                                   name=f"u{pr}", bufs=2)
                ap0 = erel.ap()
                qs = range(4) if quarter is None else [quarter]
                for i, hh in enumerate((2 * pr, 2 * pr + 1)):
                    for qq in qs:
                        shear = bass.AP(
                            tensor=ap0.tensor,
                            offset=ap0.offset + hh * NDIAG + U_OFF + qq * UQ,
                            ap=[[1, P], [1, UQ]],
                        )
                        nc.sync.dma_start(u[:, i, qq * UQ:(qq + 1) * UQ], shear)
                return u

            def load_uraw(pr, qc):
                """near-window raw-bias shear tile [P, 2, W_NEAR] for (pr, qc)."""
                w = WQC[qc]
                t = urawp.tile([P, 2, W_NEAR], DT.bfloat16, tag="uraw",
                               name=f"uraw{pr}_{qc}")
                ap0 = brel.ap()
                for i, hh in enumerate((2 * pr, 2 * pr + 1)):
                    shear = bass.AP(
                        tensor=ap0.tensor,
                        offset=ap0.offset + hh * NDIAG + B0[qc],
                        ap=[[1, P], [1, w]],
                    )
                    nc.gpsimd.dma_start(t[:, i, 0:w], shear)
                return t

            # ---------- phase B: pair-0 Q/K + V (all heads), single x^T pass ----
            for nq in range(NQC):
                qk_ps = psum.tile([P, 2 * SC], DT.float32, tag="ps",
                                  name=f"qkps0_{nq}")
                q_ps, k_ps = qk_ps[:, 0:SC], qk_ps[:, SC:2 * SC]
                v01 = psum.tile([P, 2 * SC], DT.float32, tag="ps", name=f"v01_{nq}")
                v23 = psum.tile([P, 2 * SC], DT.float32, tag="ps", name=f"v23_{nq}")
                v_ps = [v01[:, 0:SC], v01[:, SC:2 * SC],
                        v23[:, 0:SC], v23[:, SC:2 * SC]]
                for kd in range(NDT):
                    if nq == 0 and kd == 0:
                        # single-kd first chunks: the first matmul only needs
                        # wq/wk/wv[kd=0], so don't make it wait for 3MB
                        cw = HL * D_KV
                        for wsb, wsrc in ((wq_sb, wq), (wk_sb, wk), (wv_sb, wv)):
                            nc.gpsimd.dma_start(wsb[:, 0:1, :], wsrc[:, 0:cw])
                    elif nq == 0 and kd == 1:
                        cw = HL * D_KV
                        for wsb, wsrc in ((wq_sb, wq), (wk_sb, wk), (wv_sb, wv)):
                            nc.gpsimd.dma_start(wsb[:, 1:4, :],
                                                wsrc[:, cw:4 * cw])
                    elif nq == 0 and kd % 4 == 0:
                        g = kd // 4
                        c0, c1 = g * 4 * HL * D_KV, (g + 1) * 4 * HL * D_KV
                        nc.gpsimd.dma_start(wq_sb[:, g * 4:(g + 1) * 4, :],
                                            wq[:, c0:c1])
                        nc.gpsimd.dma_start(wk_sb[:, g * 4:(g + 1) * 4, :],
                                            wk[:, c0:c1])
                        nc.gpsimd.dma_start(wv_sb[:, g * 4:(g + 1) * 4, :],
                                            wv[:, c0:c1])
                    pool, eng = ((xtp, nc.sync)
                                 if (nq == 0 or kd % 2 == 0)
                                 else (xtpc, nc.gpsimd))
                    xt_t = pool.tile([P, SC], DT.bfloat16, tag="xt",
                                     name=f"xb{nq}_{kd}")
                    eng.dma_start(
                        xt_t[:], xt[kd * P:(kd + 1) * P, nq * SC:(nq + 1) * SC]
                    )
                    nc.tensor.matmul(
                        q_ps, wq_sb[:, kd, 0:P], xt_t[:],
                        start=(kd == 0), stop=(kd == NDT - 1),
                    )
                    nc.tensor.matmul(
                        k_ps, wk_sb[:, kd, 0:P], xt_t[:],
                        start=(kd == 0), stop=(kd == NDT - 1),
                    )
                    for st in range(4):
                        nc.tensor.matmul(
                            v_ps[st], xt_t[:, st * P:(st + 1) * P],
                            wv_sb[:, kd, :],
                            start=(kd == 0), stop=(kd == NDT - 1),
                        )
                if nq == 0:
                    nc.scalar.dma_start(
                        wo_sb.rearrange("p a b -> p (a b)"), wo[:])
                # drain: V -> vaug blocks first (frees the 2 V psum slots the
                # next nq's V matmuls are waiting on), then q/k casts
                for st in range(4):
                    ktg = nq * 4 + st
                    vsrc = v_ps[st].rearrange("p (pr par d) -> p pr par d",
                                              par=2, d=D_KV)
                    nc.vector.tensor_copy(vaug[:, ktg, :, 0:D_KV],
                                          vsrc[:, :, 0, :])
                    nc.vector.tensor_copy(vaug[:, ktg, :, 129:193],
                                          vsrc[:, :, 1, :])
                nc.vector.tensor_copy(rev_ap(qt_sb[:, 0, :], nq * SC), q_ps)
                nc.vector.tensor_copy(kt_sb[:, 0, nq * SC:(nq + 1) * SC], k_ps)

            # ---------- phase C: attention, proj of pair pr+1 interleaved ----
            def emit_sg(pr, qc, kt, uraw_t):
                """scores psum group for (pair, q-chunk, k-tile), with the
                Toeplitz bias identity-injected first on near tiles."""
                jg0 = qc * SC
                s01 = psum.tile([P, 2 * SC], DT.float32, tag="ps",
                                name=f"s{pr}_{qc}_{kt}")
                near = NEAR[(kt, qc)]
                j0 = kt * P + jg0
                if near:
                    a = j0 - B0[qc]
                    nc.tensor.matmul(
                        s01[:, 0:SC], id_sb[:], uraw_t[:, 0, a:a + SC],
                        start=True, stop=False,
                    )
                    nc.tensor.matmul(
                        s01[:, SC:2 * SC], id_sb[:], uraw_t[:, 1, a:a + SC],
                        start=True, stop=False,
                    )
                nc.tensor.matmul(
                    s01[:, 0:SC], kt_sb[0:64, pr, kt * P:(kt + 1) * P],
                    qt_sb[0:64, pr, jg0:jg0 + SC],
                    start=not near, stop=True, tile_position=(0, 0),
                )
                nc.tensor.matmul(
                    s01[:, SC:2 * SC], kt_sb[64:128, pr, kt * P:(kt + 1) * P],
                    qt_sb[64:128, pr, jg0:jg0 + SC],
                    start=not near, stop=True, tile_position=(64, 0),
                )
                return s01

            def attn_qc(pr, qc, u_t, uraw_t, proj, pending, s_pre, nxt_sg):
                """attention for head pair pr, reversed-q chunk qc.
                proj: None or pr+1 (emit that pair's Q/K proj, 1 kd per kt).
                Emission order per kt puts all independent PE work BEFORE the
                dependent PV matmuls so the in-order PE queue can fill
                ACT-wait bubbles."""
                h0, h1 = 2 * pr, 2 * pr + 1
                jg0 = qc * SC
                cx01 = psum.tile([P, 2 * SC], DT.float32, tag="ps",
                                 name=f"cx{pr}_{qc}")
                if proj is not None:
                    pj_ps = psum.tile([P, 2 * SC], DT.float32, tag="ps",
                                      name=f"pjps{proj}_{qc}")
                    pjq, pjk = pj_ps[:, 0:SC], pj_ps[:, SC:2 * SC]

                def emit_proj(kd):
                    pool, eng = ((xtp, nc.sync) if kd % 2 == 0
                                 else (xtpc, nc.gpsimd))
                    xt_t = pool.tile([P, SC], DT.bfloat16, tag="xt",
                                     name=f"xp{proj}_{qc}_{kd}")
                    eng.dma_start(
                        xt_t[:], xt[kd * P:(kd + 1) * P, jg0:jg0 + SC]
                    )
                    nc.tensor.matmul(
                        pjq, wq_sb[:, kd, proj * P:(proj + 1) * P], xt_t[:],
                        start=(kd == 0), stop=(kd == NDT - 1),
                    )
                    nc.tensor.matmul(
                        pjk, wk_sb[:, kd, proj * P:(proj + 1) * P], xt_t[:],
                        start=(kd == 0), stop=(kd == NDT - 1),
                    )

                # 2-deep software pipeline: s(kt+2) is emitted before PV(kt)
                # so the in-order PE queue keeps a backlog (hides LDWEIGHTS
                # and cross-engine semaphore latency).  pending() emits the
                # PREVIOUS qc's deferred normalize chain (DVE+DMA only).
                sq = [s_pre if s_pre is not None else emit_sg(pr, qc, 0, uraw_t),
                      emit_sg(pr, qc, 1, uraw_t)]
                for kt in range(NKT):
                    if kt + 2 < NKT:
                        sq.append(emit_sg(pr, qc, kt + 2, uraw_t))
                    if proj is not None:
                        emit_proj(kt)
                    if kt == 2 and pending is not None:
                        pending()
                    s01 = sq[kt]
                    px = pexpp.tile([P, 2 * SC], DT.bfloat16, tag="pexp",
                                    name=f"px{pr}_{qc}_{kt}")
                    nc.scalar.activation(
                        out=px[:], in_=s01[:], func=AF.Exp,
                        bias=mask_sb[:, kt:kt + 1], scale=1.0 / math.sqrt(D_KV),
                    )
                    if not NEAR[(kt, qc)]:
                        j0 = kt * P + jg0 - U_OFF
                        nc.vector.tensor_tensor(
                            px.rearrange("p (h j) -> p h j", h=2),
                            px.rearrange("p (h j) -> p h j", h=2),
                            u_t[:, :, j0:j0 + SC], OP.mult
                        )
                    nc.tensor.matmul(
                        cx01[0:65, 0:SC], vaug[:, kt, pr, 0:65], px[:, 0:SC],
                        start=(kt == 0), stop=(kt == NKT - 1),
                    )
                    nc.tensor.matmul(
                        cx01[:, SC:2 * SC], vaug[:, kt, pr, 65:VW],
                        px[:, SC:2 * SC],
                        start=(kt == 0), stop=(kt == NKT - 1),
                    )

                # proj drain (reversed q for qt)
                if proj is not None:
                    nc.vector.tensor_copy(rev_ap(qt_sb[:, proj, :], jg0), pjq)
                    nc.vector.tensor_copy(
                        kt_sb[:, proj, jg0:jg0 + SC], pjk)

                # pre-emit the NEXT qc's first score group so ACT never idles
                # across the boundary (exactly one PSUM slot is free here)
                s_next = nxt_sg() if nxt_sg is not None else None

                # ---- evacuate cx to SBUF (frees the PSUM slot), then the
                # rest of normalize+writeback is DEFERRED into the next qc
                # (DVE + DMA only; the PE never touches it) ----
                cxs = stage.tile([P, 2 * SC], DT.bfloat16, tag="cxs",
                                 name=f"cxs{pr}_{qc}", bufs=1)
                nc.vector.tensor_copy(cxs[:], cx01[:])

                def normalize():
                    # denominators: h0 on row 64 (cols 0:512), h1 on row 32
                    # (cols 512:1024).  Custom DVE ops need base-partition-0
                    # operands, so pack both rows into a base-0 tile first.
                    dnf = stage.tile([P, SC], DT.float32, tag="dnf",
                                     name=f"dnf{pr}_{qc}", bufs=1)
                    nc.vector.tensor_copy(dnf[64:65, :], cxs[64:65, 0:SC])
                    nc.vector.tensor_copy(dnf[32:33, :], cxs[32:33, SC:2 * SC])
                    rb = stage.tile([P, SC], DT.float32, tag="rb",
                                    name=f"rb{pr}_{qc}", bufs=1)
                    nc.vector.reciprocal_approx_fast(out=rb[:], in_=dnf[:])
                    rbh = stage.tile([P, SC], DT.bfloat16, tag="rbh",
                                     name=f"rbh{pr}_{qc}", bufs=1)
                    nc.vector.tensor_copy(rbh[64:65, :], rb[64:65, :])
                    nc.vector.tensor_copy(rbh[32:33, :], rb[32:33, :])
                    # broadcast across partitions: bounce the two reciprocal
                    # rows through DRAM, then stride-0 DMA reads replicate
                    # them to 64 partitions each (all off the engine queues).
                    bnc = dramp.tile([2, SC], DT.bfloat16, tag="bnc",
                                     name=f"bnc{pr}_{qc}")
                    nc.gpsimd.dma_start(bnc[0:1, :], rbh[64:65, :])
                    nc.gpsimd.dma_start(bnc[1:2, :], rbh[32:33, :])
                    bc_sb = stage.tile([P, SC], DT.bfloat16, tag="bc",
                                       name=f"bcs{pr}_{qc}", bufs=1)
                    src0 = bass.AP(tensor=bnc.tensor, offset=bnc.offset,
                                   ap=[[0, 64], [1, SC]])
                    src1 = bass.AP(tensor=bnc.tensor, offset=bnc.offset + SC,
                                   ap=[[0, 64], [1, SC]])
                    nc.gpsimd.dma_start(bc_sb[0:64, :], src0)
                    nc.gpsimd.dma_start(bc_sb[64:128, :], src1)
                    nc.vector.tensor_tensor(
                        rev_ap(ctxt[0:64, pr, :], jg0),
                        cxs[0:64, 0:SC], bc_sb[0:64, :], OP.mult)
                    nc.vector.tensor_tensor(
                        rev_ap(ctxt[64:128, pr, :], jg0),
                        cxs[64:128, SC:2 * SC], bc_sb[64:128, :], OP.mult)
                return normalize, s_next

            def emit_outproj_st(st, in_c=False):
                """output projection for one s-tile.  in_c=True targets the
                attention phase: single-psum-slot halves, copies on DVE only
                (ACT is the bottleneck engine there)."""
                for half in range(2):
                    o2 = psum.tile([P, 2 * SC], DT.float32, tag="ps",
                                   name=f"oc{st}_{half}")
                    oh = [o2[:, 0:SC], o2[:, SC:2 * SC]]
                    for m in range(NMT):
                        for j in range(2):
                            nd = 2 * half + j
                            nc.tensor.matmul(
                                oh[j], ctxt[:, m, st * P:(st + 1) * P],
                                wo_sb[:, m, nd * SC:(nd + 1) * SC],
                                start=(m == 0), stop=(m == NMT - 1),
                            )
                    o_t = outp.tile([P, 2, SC], DT.float32, tag="out",
                                    name=f"otc{st}_{half}")
                    if in_c:
                        nc.vector.tensor_copy(o_t[:, 0, :], oh[0])
                        nc.vector.tensor_copy(o_t[:, 1, :], oh[1])
                    else:
                        nc.scalar.copy(o_t[:, 0, :], oh[0])
                        nc.vector.tensor_copy(o_t[:, 1, :], oh[1])
                    nc.sync.dma_start(
                        out[st * P:(st + 1) * P,
                            half * 2 * SC:(half + 1) * 2 * SC],
                        o_t[:],
                    )

            u_t = load_u(0)
            uraw_next = load_uraw(0, 0)
            pending = None
            s_pre = None
            seq = [(pr, qc) for pr in range(NPAIR) for qc in range(NQC)]
            for idx, (pr, qc) in enumerate(seq):
                nxt = pr + 1 if pr + 1 < NPAIR else None
                uraw_t = uraw_next
                # prefetch next (pair, qc) raw window
                if qc + 1 < NQC:
                    uraw_next = load_uraw(pr, qc + 1)
                elif nxt is not None:
                    uraw_next = load_uraw(nxt, 0)
                if nxt is not None:
                    next_u = load_u(nxt, quarter=qc,
                                    u=None if qc == 0 else next_u)
                if idx + 1 < len(seq):
                    npr, nqc = seq[idx + 1]
                    un = uraw_next
                    nxt_sg = (lambda npr=npr, nqc=nqc, un=un:
                              emit_sg(npr, nqc, 0, un))
                else:
                    nxt_sg = None
                pending, s_pre = attn_qc(pr, qc, u_t, uraw_t, nxt, pending,
                                         s_pre, nxt_sg)
                if qc == NQC - 1 and nxt is not None:
                    u_t = next_u
            pending()

            # ---------- phase D: output projection (descending st: the
            # low-st tiles depend on the last deferred normalize) ----------
            for st in range(NKT - 1, -1, -1):
                oa = psum.tile([P, 2 * SC], DT.float32, tag="ps",
                               name=f"oa{st}")
                ob = psum.tile([P, 2 * SC], DT.float32, tag="ps",
                               name=f"ob{st}")
                o_ps = [oa[:, 0:SC], oa[:, SC:2 * SC],
                        ob[:, 0:SC], ob[:, SC:2 * SC]]
                for m in range(NMT):
                    for nd in range(NQC):
                        nc.tensor.matmul(
                            o_ps[nd], ctxt[:, m, st * P:(st + 1) * P],
                            wo_sb[:, m, nd * SC:(nd + 1) * SC],
                            start=(m == 0), stop=(m == NMT - 1),
                        )
                for half in range(2):
                    o_t = outp.tile([P, 2, SC], DT.float32, tag="out",
                                    name=f"ot{st}_{half}")
                    nc.scalar.copy(o_t[:, 0, :], o_ps[2 * half])
                    nc.vector.tensor_copy(o_t[:, 1, :], o_ps[2 * half + 1])
                    nc.sync.dma_start(
                        out[st * P:(st + 1) * P,
                            half * 2 * SC:(half + 1) * 2 * SC],
                        o_t[:],
                    )

    nc.finalize()
    return nc


_NC_CACHE = None


def _get_nc():
    global _NC_CACHE
    if _NC_CACHE is None:
        _NC_CACHE = _build()
    return _NC_CACHE


def _in_maps(hidden_states, attention_mask, Wq, Wk, Wv, Wo, rel_emb):
    import ml_dtypes
    bf16 = ml_dtypes.bfloat16
    ident = np.eye(P, dtype=np.float32).astype(bf16)
    maps = []
    for c in range(NCORES):
        b, g = c // 4, c % 4
        hlo, hhi = g * HL, (g + 1) * HL
        brel, erel = _bias_tables(
            np.ascontiguousarray(rel_emb[:, hlo:hhi], dtype=np.float32))
        def shuf(w):  # [NDT*P, C] -> [P, NDT*C] partition-contiguous
            c = w.shape[1]
            return np.ascontiguousarray(
                w.reshape(-1, P, c).transpose(1, 0, 2).reshape(P, -1))
        maps.append({
            "xt": np.ascontiguousarray(hidden_states[b].T).astype(bf16),
            "wq": shuf(Wq[:, hlo * D_KV:hhi * D_KV]).astype(bf16),
            "wk": shuf(Wk[:, hlo * D_KV:hhi * D_KV]).astype(bf16),
            "wv": shuf(Wv[:, hlo * D_KV:hhi * D_KV]).astype(bf16),
            "wo": shuf(Wo[hlo * D_KV:hhi * D_KV, :]).astype(bf16),
            "mask": np.ascontiguousarray(attention_mask[b, 0, 0, :]).astype(np.float32),
            "brel": brel,
            "erel": erel,
            "ident": ident,
        })
    return maps


def kernel(hidden_states, attention_mask, Wq, Wk, Wv, Wo, rel_emb, _trace=False,
           _trace_kwargs=None):
    hidden_states = np.asarray(hidden_states, dtype=np.float32)
    attention_mask = np.asarray(attention_mask, dtype=np.float32)
    Wq = np.asarray(Wq, dtype=np.float32)
    Wk = np.asarray(Wk, dtype=np.float32)
    Wv = np.asarray(Wv, dtype=np.float32)
    Wo = np.asarray(Wo, dtype=np.float32)
    rel_emb = np.asarray(rel_emb, dtype=np.float32)

    nc = _get_nc()
    maps = _in_maps(hidden_states, attention_mask, Wq, Wk, Wv, Wo, rel_emb)
    kw = dict(_trace_kwargs or {})
    res = run_bass_kernel_spmd(nc, maps, core_ids=list(range(NCORES)),
                               trace=_trace, **kw)
    kernel.last_results = res
    outp = np.empty((B, S, D), dtype=np.float32)
    for b in range(B):
        acc = np.asarray(res.results[4 * b]["out"], dtype=np.float32).copy()
        for g in range(1, 4):
            acc += np.asarray(res.results[4 * b + g]["out"], dtype=np.float32)
        outp[b] = acc
    return outp

